# revision 3
# baseline (speedup 1.0000x reference)
"""BOW window features kernel for Trainium2 (8 NeuronCores, SPMD).

Problem (hardcoded): tokens [16, 1024] int32 in [0, 2048) ->
out [16, 1024, 5*2048] f32 where
  out[b, m, k*2048 + tokens[b, m - (k-2)]] = 1   for k in 0..4, 0 <= m-(k-2) < 1024
and 0 elsewhere.

Strategy: data-parallel over batch (2 rows/core). Per core, for each chunk of
128 source positions, compute the one-hot block [128, 2048] on the vector
engine (iota == token per-partition compare), then DMA the same SBUF block to
the 5 diagonal window-slot destinations in DRAM. Uncovered boundary rows
(6 per batch row) are zero-filled from a zeros tile. Output writes total
80 MB/core -> HBM-write bound.
"""

import numpy as np

B = 16
L = 1024
NT = 2048
W = 2
K = 2 * W + 1
P = 128
NCORES = 8
BPC = B // NCORES  # batch rows per core
CH = L // P        # position chunks per batch row

_CACHE = {}


def _build_nc():
    import concourse.bacc as bacc
    import concourse.mybir as mybir
    from concourse import tile

    nc = bacc.Bacc("TRN2", debug=False)
    tokens = nc.dram_tensor("tokens", [BPC, L], mybir.dt.int32, kind="ExternalInput")
    out = nc.dram_tensor("out", [BPC, L, K * NT], mybir.dt.float32, kind="ExternalOutput")

    with tile.TileContext(nc) as tc:
        with (
            tc.tile_pool(name="const", bufs=1) as cpool,
            tc.tile_pool(name="oh", bufs=4) as ohpool,
        ):
            # Token ids < 2048 are exact in f32, so compare in f32 (the
            # vector-engine is_equal path requires an f32 scalar operand).
            iota_t = cpool.tile([P, NT], mybir.dt.float32)
            nc.gpsimd.iota(
                iota_t[:], [[1, NT]], channel_multiplier=0,
                allow_small_or_imprecise_dtypes=True,
            )

            zeros_t = cpool.tile([P, NT], mybir.dt.float32)
            nc.gpsimd.memset(zeros_t[:], 0.0)

            # tok_i[p, b*CH + t] = tokens[b, t*128 + p]
            tok_i = cpool.tile([P, BPC * CH], mybir.dt.int32)
            nc.sync.dma_start(
                out=tok_i[:].rearrange("p (b t) -> p b t", b=BPC),
                in_=tokens[:].rearrange("b (t p) -> p b t", p=P),
            )
            tok_t = cpool.tile([P, BPC * CH], mybir.dt.float32)
            nc.scalar.copy(tok_t[:], tok_i[:])

            for b in range(BPC):
                # Boundary rows never written by the shifted copies: zero them.
                for k in range(K):
                    i = k - W  # out position m is fed by source n = m - i
                    if i > 0:
                        nc.sync.dma_start(
                            out=out[b, 0:i, k * NT:(k + 1) * NT],
                            in_=zeros_t[0:i, :],
                        )
                    elif i < 0:
                        nc.sync.dma_start(
                            out=out[b, L + i:L, k * NT:(k + 1) * NT],
                            in_=zeros_t[0:-i, :],
                        )

                for t in range(CH):
                    oh = ohpool.tile([P, NT], mybir.dt.float32)
                    nc.vector.tensor_scalar(
                        out=oh[:],
                        in0=iota_t[:],
                        scalar1=tok_t[:, b * CH + t: b * CH + t + 1],
                        scalar2=None,
                        op0=mybir.AluOpType.is_equal,
                    )
                    n0 = t * P
                    for k in range(K):
                        i = k - W
                        m0 = n0 + i
                        lo, hi = max(m0, 0), min(m0 + P, L)
                        nc.sync.dma_start(
                            out=out[b, lo:hi, k * NT:(k + 1) * NT],
                            in_=oh[lo - m0: hi - m0, :],
                        )
    nc.compile()
    return nc


def _get_nc():
    if "nc" not in _CACHE:
        _CACHE["nc"] = _build_nc()
    return _CACHE["nc"]


def run_spmd(tokens: np.ndarray, trace: bool = False):
    """Run on 8 cores; returns (out [16, 1024, K*NT] f32, BassKernelResults)."""
    from concourse.bass_utils import run_bass_kernel_spmd

    tokens = np.ascontiguousarray(np.asarray(tokens, dtype=np.int32))
    assert tokens.shape == (B, L)
    nc = _get_nc()
    in_maps = [
        {"tokens": np.ascontiguousarray(tokens[c * BPC:(c + 1) * BPC])}
        for c in range(NCORES)
    ]
    res = run_bass_kernel_spmd(nc, in_maps, list(range(NCORES)), trace=trace)
    out = np.concatenate([res.results[c]["out"] for c in range(NCORES)], axis=0)
    return out.reshape(B, L, K * NT), res


def kernel(tokens: np.ndarray) -> np.ndarray:
    out, _ = run_spmd(tokens, trace=False)
    return out


# revision 4
# speedup vs baseline: 1.5408x; 1.5408x over previous
"""BOW window features kernel for Trainium2 (8 NeuronCores, SPMD).

Problem (hardcoded): tokens [16, 1024] int32 in [0, 2048) ->
out [16, 1024, 5*2048] f32 where
  out[b, m, k*2048 + tokens[b, m - (k-2)]] = 1   for k in 0..4, 0 <= m-(k-2) < 1024
and 0 elsewhere.

Strategy: data-parallel over batch (2 rows/core). Per core, for each chunk of
128 source positions, compute the one-hot block [128, 2048] on the vector
engine (iota == token per-partition compare), then DMA the same SBUF block to
the 5 diagonal window-slot destinations in DRAM. Uncovered boundary rows
(6 per batch row) are zero-filled from a zeros tile. Output writes total
80 MB/core -> HBM-write bound.
"""

import numpy as np

B = 16
L = 1024
NT = 2048
W = 2
K = 2 * W + 1
P = 128
NCORES = 8
BPC = B // NCORES  # batch rows per core
CH = L // P        # position chunks per batch row

_CACHE = {}


def _build_nc():
    import concourse.bacc as bacc
    import concourse.mybir as mybir
    from concourse import tile

    nc = bacc.Bacc("TRN2", debug=False)
    tokens = nc.dram_tensor("tokens", [BPC, L], mybir.dt.int32, kind="ExternalInput")
    out = nc.dram_tensor("out", [BPC, L, K * NT], mybir.dt.float32, kind="ExternalOutput")

    with tile.TileContext(nc) as tc:
        with (
            tc.tile_pool(name="const", bufs=1) as cpool,
            tc.tile_pool(name="oh", bufs=4) as ohpool,
        ):
            # Token ids < 2048 are exact in f32, so compare in f32 (the
            # vector-engine is_equal path requires an f32 scalar operand).
            iota_t = cpool.tile([P, NT], mybir.dt.float32)
            nc.gpsimd.iota(
                iota_t[:], [[1, NT]], channel_multiplier=0,
                allow_small_or_imprecise_dtypes=True,
            )

            zeros_t = cpool.tile([P, NT], mybir.dt.float32)
            nc.gpsimd.memset(zeros_t[:], 0.0)

            # tok_i[p, b*CH + t] = tokens[b, t*128 + p]
            tok_i = cpool.tile([P, BPC * CH], mybir.dt.int32)
            nc.sync.dma_start(
                out=tok_i[:].rearrange("p (b t) -> p b t", b=BPC),
                in_=tokens[:].rearrange("b (t p) -> p b t", p=P),
            )
            tok_t = cpool.tile([P, BPC * CH], mybir.dt.float32)
            nc.scalar.copy(tok_t[:], tok_i[:])

            for b in range(BPC):
                # Boundary rows never written by the shifted copies: zero them.
                for k in range(K):
                    i = k - W  # out position m is fed by source n = m - i
                    if i > 0:
                        nc.sync.dma_start(
                            out=out[b, 0:i, k * NT:(k + 1) * NT],
                            in_=zeros_t[0:i, :],
                        )
                    elif i < 0:
                        nc.sync.dma_start(
                            out=out[b, L + i:L, k * NT:(k + 1) * NT],
                            in_=zeros_t[0:-i, :],
                        )

                for t in range(CH):
                    oh = ohpool.tile([P, NT], mybir.dt.float32)
                    nc.vector.tensor_scalar(
                        out=oh[:],
                        in0=iota_t[:],
                        scalar1=tok_t[:, b * CH + t: b * CH + t + 1],
                        scalar2=None,
                        op0=mybir.AluOpType.is_equal,
                    )
                    n0 = t * P
                    for k in range(K):
                        i = k - W
                        m0 = n0 + i
                        lo, hi = max(m0, 0), min(m0 + P, L)
                        # Odd-partition-count DMAs collapse onto a single SDMA
                        # engine (observed: 127-row transfers land 100% on one
                        # engine and serialize at ~26 GB/s). Emit even-count
                        # transfers only; peel off one row when odd.
                        if (hi - lo) % 2 == 1:
                            nc.sync.dma_start(
                                out=out[b, lo:lo + 1, k * NT:(k + 1) * NT],
                                in_=oh[lo - m0: lo - m0 + 1, :],
                            )
                            lo += 1
                        if hi > lo:
                            nc.sync.dma_start(
                                out=out[b, lo:hi, k * NT:(k + 1) * NT],
                                in_=oh[lo - m0: hi - m0, :],
                            )
    nc.compile()
    return nc


def _get_nc():
    if "nc" not in _CACHE:
        _CACHE["nc"] = _build_nc()
    return _CACHE["nc"]


def run_spmd(tokens: np.ndarray, trace: bool = False):
    """Run on 8 cores; returns (out [16, 1024, K*NT] f32, BassKernelResults)."""
    from concourse.bass_utils import run_bass_kernel_spmd

    tokens = np.ascontiguousarray(np.asarray(tokens, dtype=np.int32))
    assert tokens.shape == (B, L)
    nc = _get_nc()
    in_maps = [
        {"tokens": np.ascontiguousarray(tokens[c * BPC:(c + 1) * BPC])}
        for c in range(NCORES)
    ]
    res = run_bass_kernel_spmd(nc, in_maps, list(range(NCORES)), trace=trace)
    out = np.concatenate([res.results[c]["out"] for c in range(NCORES)], axis=0)
    return out.reshape(B, L, K * NT), res


def kernel(tokens: np.ndarray) -> np.ndarray:
    out, _ = run_spmd(tokens, trace=False)
    return out
